# revision 6
# baseline (speedup 1.0000x reference)
"""Trainium2 Bass kernel for the BottleneckIndependent MoE-routed conv block.

Math (per sample b):
  rw1 = sigmoid(mean_hw(x) @ r1_w + r1_b)                     [E]
  cw1 = sum_e rw1[e] * w1[e]          (per-sample 1x1 weights)
  out1 = relu(bn1(cw1 @ x))
  rw2 / cw2 / out2: same with 3x3 conv (pad 1)
  rw3 / cw3: 1x1; out = relu(bn3(cw3 @ out2) + x)

Strategy (8 cores, data-parallel over batch, 4 samples/core):
  * BN scales fold into expert weights on the host; BN bias + ReLU fuse into
    one ScalarE/VectorE epilogue op per output chunk.  All device tensors are
    fp16 (same bytes as bf16, 8x less rounding noise).
  * The rank-8 expert combine runs on the PE with the expert weights as the
    STATIONARY operand ([128,128] chunks, rows=(j,e)) against a block-diagonal
    routing matrix bd[128, 64] (cols=(b,j)), yielding combined weights directly
    in [i_partition, ...] layout -- the lhsT layout the conv matmuls need.
  * Single-queue FIFO DMA: every input rides the sync (SP) HWDGE queue in
    consumption order x -> cbf/cf32 -> w1 -> w2 -> w3 with no cross-queue
    dependency chains, so the ~435 GB/s per-core port never idles between
    tensors.  Output writebacks also ride sync (in-order after w3's dispatch).
  * Convs are bank-paced: weight chunk order in DRAM is arranged so each
    combine bank (8 chunks -> one PSUM tile) unblocks exactly the conv matmuls
    that consume it; conv work trails the weight DMA by ~1 bank instead of a
    whole oc-half.  Stage-3 (combine3+conv3+writeback) pipelines per output
    chunk behind w3's 4 slices.
  * The 3x3 is 9 shifted 1x1 matmuls accumulating in PSUM over a zero-padded
    16x16 buffer.  The residual add is an identity matmul into the conv3
    PSUM group.  Pool/bias epilogues alternate ACT/DVE; PSUM->SBUF combine
    copies rotate DVE/ACT/GPSIMD.
"""

import numpy as np

B, INP, WIDTH, OUTP, E, H = 32, 1024, 256, 1024, 8, 14
EPS = 1e-5
S = H * H            # 196
SP = 256             # 16*16 padded spatial
NCORES = 8
BS = B // NCORES     # 4 samples per core
P = 128

F16 = np.float16

N_CHUNKS1 = 2 * 8 * 8        # stage1: c1 = (oc*8 + ic)*8 + gl   [bank=(oc,ic)]
N_CHUNKS2 = 2 * 9 * 2 * 8    # stage2: c2 = gh*144+tap*16+ic*8+gl [bank=(gh,tap,ic)]
N_CHUNKS3 = 8 * 2 * 8        # stage3: c3 = (oc*2 + ic)*8 + gl   [bank=(oc,ic)]

_nc_cache = None
last_exec_time_ns = None
last_trace_path = None
last_res = None


# ----------------------------------------------------------------------------
# Host-side input preparation (pure numpy)
# ----------------------------------------------------------------------------

def _fold_bn(g, b, m, v):
    inv = (g / np.sqrt(v + EPS)).astype(np.float32)
    beta = (b - m * inv).astype(np.float32)
    return inv, beta


def _prep_weights(w1, w2, w3, r1_w, r1_b, r2_w, r2_b, r3_w, r3_b,
                  bn1_g, bn1_b, bn1_m, bn1_v, bn2_g, bn2_b, bn2_m, bn2_v,
                  bn3_g, bn3_b, bn3_m, bn3_v):
    inv1, beta1 = _fold_bn(bn1_g, bn1_b, bn1_m, bn1_v)
    inv2, beta2 = _fold_bn(bn2_g, bn2_b, bn2_m, bn2_v)
    inv3, beta3 = _fold_bn(bn3_g, bn3_b, bn3_m, bn3_v)

    w1p = (w1[:, :, :, 0, 0] * inv1[None, :, None]).astype(np.float32)  # [E,256,1024]
    w2p = (w2 * inv2[None, :, None, None, None]).astype(np.float32)     # [E,256,256,3,3]
    w3p = (w3[:, :, :, 0, 0] * inv3[None, :, None]).astype(np.float32)  # [E,1024,256]

    # stage1 rows (j,e); chunk c1 = (oc*8+ic)*8+gl; o = (oc*8+gl)*16+j
    a = w1p.reshape(E, 2, 8, 16, 8, P)                  # e, oc, gl, j, ic, ip
    w1r = a.transpose(3, 0, 1, 4, 2, 5).reshape(P, N_CHUNKS1 * P).astype(F16)

    # stage2 chunk c2 = gh*144 + tap*16 + ic*8 + gl; g = gh*8+gl
    a = w2p.reshape(E, 2, 8, 16, 2, P, 3, 3)            # e, gh, gl, j, ic, ip, kh, kw
    w2r = a.transpose(3, 0, 1, 6, 7, 4, 2, 5).reshape(P, N_CHUNKS2 * P).astype(F16)

    # stage3 chunk c3 = (oc*2+ic)*8+gl; o = (oc*8+gl)*16+j
    a = w3p.reshape(E, 8, 8, 16, 2, P)                  # e, oc, gl, j, ic, ip
    w3r = a.transpose(3, 0, 1, 4, 2, 5).reshape(P, N_CHUNKS3 * P).astype(F16)

    def rep_routing(rw, nchunks):
        # [C, E] -> [128, nchunks*128]; col m of chunk ic = rw[ic*128+p, m%8]/S
        r = (np.asarray(rw, np.float32) / float(S)).reshape(nchunks, P, E)
        rrep = np.tile(r[:, :, None, :], (1, 1, 16, 1)).reshape(nchunks, P, P)
        return rrep.transpose(1, 0, 2).reshape(P, nchunks * P)

    # fp16 constant pack: mask(64) | ident(128) | r1rep(1024) | r2rep(256) | r3rep(256)
    jj = np.arange(P)[:, None] // 8
    col_j = np.tile(np.arange(16), 4)[None, :]
    mask = (col_j == jj).astype(np.float32)
    ident = np.eye(P, dtype=np.float32)
    cbf = np.concatenate(
        [mask, ident, rep_routing(r1_w, 8), rep_routing(r2_w, 2),
         rep_routing(r3_w, 2)], axis=1).astype(F16)    # [128, 1728]

    # f32 constant pack: rb(3) | beta1(2) | beta2(2) | beta3(8)
    rb = np.stack([np.tile(np.asarray(r, np.float32), 16)
                   for r in (r1_b, r2_b, r3_b)], axis=1)
    cf32 = np.concatenate(
        [rb, beta1.reshape(2, P).T, beta2.reshape(2, P).T,
         beta3.reshape(8, P).T], axis=1).astype(np.float32)  # [128, 15]

    return dict(w1r=w1r, w2r=w2r, w3r=w3r, cbf=cbf, cf32=cf32)


def _prep_x(x):
    out = []
    for c in range(NCORES):
        xc = np.asarray(x[c * BS:(c + 1) * BS], np.float32)
        xb = xc.reshape(BS, 8, P, S).transpose(2, 0, 1, 3).reshape(P, BS * 8 * S)
        out.append(np.ascontiguousarray(xb.astype(F16)))
    return out


# ----------------------------------------------------------------------------
# Device program
# ----------------------------------------------------------------------------

def _build_nc():
    import concourse.tile as tile
    import concourse.mybir as mybir
    from concourse.bacc import Bacc
    from contextlib import ExitStack

    f32 = mybir.dt.float32
    f16 = mybir.dt.float16
    AF = mybir.ActivationFunctionType
    ALU = mybir.AluOpType
    AX = mybir.AxisListType

    nc = Bacc("TRN2")

    xd = nc.dram_tensor("x_f16", [P, BS * 8 * S], f16, kind="ExternalInput")
    w1d = nc.dram_tensor("w1r", [P, N_CHUNKS1 * P], f16, kind="ExternalInput")
    w2d = nc.dram_tensor("w2r", [P, N_CHUNKS2 * P], f16, kind="ExternalInput")
    w3d = nc.dram_tensor("w3r", [P, N_CHUNKS3 * P], f16, kind="ExternalInput")
    cbfd = nc.dram_tensor("cbf", [P, 1728], f16, kind="ExternalInput")
    cf32d = nc.dram_tensor("cf32", [P, 15], f32, kind="ExternalInput")
    # oc-major output: [P, (oc, b, s)]
    outd = nc.dram_tensor("out", [P, 8 * BS * S], f16, kind="ExternalOutput")

    with tile.TileContext(nc) as tc, ExitStack() as ctx:
        singles = ctx.enter_context(tc.tile_pool(name="singles", bufs=1))
        wbig = ctx.enter_context(tc.tile_pool(name="wbig", bufs=1))
        cwa = ctx.enter_context(tc.tile_pool(name="cwa", bufs=1))
        ostage = ctx.enter_context(tc.tile_pool(name="ostage", bufs=2))
        kpsum = ctx.enter_context(tc.tile_pool(name="kpsum", bufs=3, space="PSUM"))
        cpsum = ctx.enter_context(tc.tile_pool(name="cpsum", bufs=4, space="PSUM"))
        rpsum = ctx.enter_context(tc.tile_pool(name="rpsum", bufs=1, space="PSUM"))

        # ---- single-queue DMA plan: x, cbf, cf32, w1, w2 (w3 after stage1) --
        x_sb = singles.tile([P, BS * 8 * S], f16)
        for sl in range(2):
            w = BS * 8 * S // 2
            nc.sync.dma_start(out=x_sb[:, sl * w:(sl + 1) * w],
                              in_=xd[:, sl * w:(sl + 1) * w])
        cbf_sb = singles.tile([P, 1728], f16)
        nc.sync.dma_start(out=cbf_sb, in_=cbfd[:, :])
        cf32_sb = singles.tile([P, 15], f32)
        nc.sync.dma_start(out=cf32_sb, in_=cf32d[:, :])

        mask_sb = cbf_sb[:, 0:64]
        ident_sb = cbf_sb[:, 64:192]
        r1w_sb = cbf_sb[:, 192:1216]
        r2w_sb = cbf_sb[:, 1216:1472]
        r3w_sb = cbf_sb[:, 1472:1728]
        rb_sb = cf32_sb[:, 0:3]
        beta_sb = cf32_sb[:, 3:15]

        w1_sb = wbig.tile([P, N_CHUNKS1 * P], f16, tag="wbig", name="w1_sb")
        for sl in range(4):
            w = N_CHUNKS1 * P // 4
            nc.sync.dma_start(out=w1_sb[:, sl * w:(sl + 1) * w],
                              in_=w1d[:, sl * w:(sl + 1) * w])
        w2_sb = singles.tile([P, N_CHUNKS2 * P], f16)
        for sl in range(6):
            w = N_CHUNKS2 * P // 6
            nc.sync.dma_start(out=w2_sb[:, sl * w:(sl + 1) * w],
                              in_=w2d[:, sl * w:(sl + 1) * w])

        # ---- working tiles ----------------------------------------------
        cw1 = cwa.tile([P, BS * 8 * 2 * P], f16, tag="cwa", name="cw1")
        cw2 = singles.tile([P, BS * 9 * 2 * 2 * P], f16)
        out1pad = singles.tile([P, BS * 2 * SP], f16)
        nc.vector.memset(out1pad, 0.0)
        out2 = singles.tile([P, BS * 2 * S], f16)

        pool1 = singles.tile([P, 8 * BS], f32)
        pool1b = singles.tile([P, 8 * BS], f16)
        pool2 = singles.tile([P, 2 * BS], f32)
        pool2b = singles.tile([P, 2 * BS], f16)
        pool3 = singles.tile([P, 2 * BS], f32)
        pool3b = singles.tile([P, 2 * BS], f16)

        x_v = x_sb.rearrange("p (b c s) -> p b c s", b=BS, c=8)
        mask_v = mask_sb.rearrange("p (b j) -> p b j", b=BS)
        out1pad_v = out1pad.rearrange("p (b c h w) -> p b c h w", b=BS, c=2, h=16)
        # cw views: cols (b, ic, oc, gl, j)
        cw1_v = cw1.rearrange("p (b ic oc gl j) -> p b ic oc gl j",
                              b=BS, ic=8, oc=2, gl=8)
        cw2_v = cw2.rearrange("p (b t ic gh gl j) -> p gh t ic gl b j",
                              b=BS, t=9, ic=2, gh=2, gl=8)

        # ---- engine rotation helpers -------------------------------------
        cp_i = [0]

        def psum_copy(dst, src):
            k = cp_i[0] % 2
            cp_i[0] += 1
            if k == 0:
                nc.vector.tensor_copy(out=dst, in_=src)
            else:
                nc.scalar.copy(dst, src)

        ep_i = [0]

        def epilogue(dst, src, bias_col):
            k = ep_i[0] % 2
            ep_i[0] += 1
            if k == 0:
                nc.scalar.activation(out=dst, in_=src, func=AF.Relu,
                                     bias=beta_sb[:, bias_col:bias_col + 1],
                                     scale=1.0)
            else:
                nc.vector.tensor_scalar(
                    out=dst, in0=src,
                    scalar1=beta_sb[:, bias_col:bias_col + 1], scalar2=0.0,
                    op0=ALU.add, op1=ALU.max)

        def pool_piece(dst, src):
            nc.vector.tensor_reduce(out=dst, in_=src, axis=AX.X, op=ALU.add)

        # ---- pooling 1 (sum over spatial; 1/S folded into routing w) -----
        pool1_v = pool1.rearrange("p (c b) -> p c b", b=BS)
        for b in range(BS):
            pool_piece(pool1_v[:, :, b], x_v[:, b])
        nc.vector.tensor_copy(out=pool1b, in_=pool1)

        # ---- routing helper ----------------------------------------------
        def routing(st, rw_sb, pool_f16, nchunks):
            ps = rpsum.tile([P, BS], f32, tag="rps", name=f"ps_rt{st}")
            for ic in range(nchunks):
                nc.tensor.matmul(ps, rw_sb[:, ic * P:(ic + 1) * P],
                                 pool_f16[:, ic * BS:(ic + 1) * BS],
                                 start=(ic == 0), stop=(ic == nchunks - 1))
            rwt = singles.tile([P, BS], f16, name=f"rwt{st}")
            nc.scalar.activation(out=rwt, in_=ps, func=AF.Sigmoid,
                                 bias=rb_sb[:, st:st + 1], scale=1.0)
            bd = singles.tile([P, BS * 16], f16, name=f"bd{st}")
            bd_v = bd.rearrange("p (b j) -> p b j", b=BS)
            nc.vector.tensor_tensor(
                out=bd_v, in0=mask_v,
                in1=rwt[:, :, None].to_broadcast((P, BS, 16)),
                op=ALU.mult)
            return bd

        def cmb_bank(st, w_sb, bd, bank):
            ps = kpsum.tile([P, 512], f32, tag="kps", name=f"ps_cmb{st}")
            for c8 in range(8):
                c = bank * 8 + c8
                nc.tensor.matmul(ps[:, c8 * 64:(c8 + 1) * 64],
                                 w_sb[:, c * P:(c + 1) * P], bd,
                                 start=True, stop=True)
            return ps

        # ================== stage 1 =======================================
        bd1 = routing(0, r1w_sb, pool1b, 8)

        # bank k = (oc, ic); combine -> copy; conv1 mm for bank k-1 (lag 1)
        conv1_ps = {}

        def conv1_mm(oc, ic):
            if ic == 0 and oc not in conv1_ps:
                conv1_ps[oc] = [cpsum.tile([P, 512], f32, tag="cps",
                                           name=f"ps_c1_{oc}_{b}")
                                for b in range(BS)]
            for b in range(BS):
                reg = conv1_ps[oc][b][:, 0:S]
                nc.tensor.matmul(
                    reg, cw1[:, ((b * 8 + ic) * 2 + oc) * P:
                             ((b * 8 + ic) * 2 + oc + 1) * P],
                    x_sb[:, b * 8 * S + ic * S:b * 8 * S + (ic + 1) * S],
                    start=(ic == 0), stop=(ic == 7))

        def conv1_fin(oc):
            for b in range(BS):
                src = conv1_ps[oc][b][:, 0:S]
                epilogue(out1pad_v[:, b, oc, 1:15, 1:15],
                         src.rearrange("p (h w) -> p h w", h=H), oc)
                pool_piece(pool2[:, oc * BS + b:oc * BS + b + 1],
                           out1pad[:, (b * 2 + oc) * SP:(b * 2 + oc + 1) * SP])

        prev1 = None
        for bank in range(16):
            oc, ic = bank // 8, bank % 8
            ps = cmb_bank(1, w1_sb, bd1, bank)
            if prev1 is not None:
                conv1_mm(*prev1)
                if prev1[1] == 7:
                    conv1_fin(prev1[0])
            psum_copy(cw1_v[:, :, ic, oc],
                      ps.rearrange("p (gl b j) -> p b gl j", gl=8, b=BS))
            prev1 = (oc, ic)
        conv1_mm(*prev1)
        conv1_fin(prev1[0])

        # ---- w3 DMA (sync queue, after w2; WAR on w1 slot via wbig tag) --
        w3_sb = wbig.tile([P, N_CHUNKS3 * P], f16, tag="wbig", name="w3_sb")
        for sl in range(4):
            w = N_CHUNKS3 * P // 4
            nc.sync.dma_start(out=w3_sb[:, sl * w:(sl + 1) * w],
                              in_=w3d[:, sl * w:(sl + 1) * w])

        nc.vector.tensor_copy(out=pool2b, in_=pool2)

        # ================== stage 2 =======================================
        bd2 = routing(1, r2w_sb, pool2b, 2)

        conv2_ps = {}

        def conv2_mm(gh, tap, ic):
            if tap == 0 and ic == 0:
                conv2_ps[gh] = [cpsum.tile([P, 512], f32, tag="cps",
                                           name=f"ps_c2_{gh}_{b}")
                                for b in range(BS)]
            k = tap * 2 + ic
            kh, kw = tap // 3, tap % 3
            for b in range(BS):
                reg = conv2_ps[gh][b][:, 0:S]
                nc.tensor.matmul(
                    reg.rearrange("p (h w) -> p h w", h=H),
                    cw2[:, (((b * 9 + tap) * 2 + ic) * 2 + gh) * P:
                        (((b * 9 + tap) * 2 + ic) * 2 + gh + 1) * P],
                    out1pad_v[:, b, ic, kh:kh + H, kw:kw + H],
                    start=(k == 0), stop=(k == 17))

        def conv2_fin(gh):
            for b in range(BS):
                src = conv2_ps[gh][b][:, 0:S]
                epilogue(out2[:, (b * 2 + gh) * S:(b * 2 + gh + 1) * S],
                         src, 2 + gh)
                pool_piece(pool3[:, gh * BS + b:gh * BS + b + 1],
                           out2[:, (b * 2 + gh) * S:(b * 2 + gh + 1) * S])

        prev2 = None
        for bank in range(36):
            gh, tap, ic = bank // 18, (bank % 18) // 2, bank % 2
            ps = cmb_bank(2, w2_sb, bd2, bank)
            if prev2 is not None:
                conv2_mm(*prev2)
                if prev2[1] == 8 and prev2[2] == 1:
                    conv2_fin(prev2[0])
            psum_copy(cw2_v[:, gh, tap, ic],
                      ps.rearrange("p (gl b j) -> p gl b j", gl=8, b=BS))
            prev2 = (gh, tap, ic)
        conv2_mm(*prev2)
        conv2_fin(prev2[0])

        nc.vector.tensor_copy(out=pool3b, in_=pool3)

        # ================== stage 3 =======================================
        bd3 = routing(2, r3w_sb, pool3b, 2)
        cw3 = cwa.tile([P, BS * 2 * 8 * P], f16, tag="cwa", name="cw3")
        cw3_v = cw3.rearrange("p (b ic oc gl j) -> p b ic oc gl j",
                              b=BS, ic=2, oc=8, gl=8)
        outd_v = outd.rearrange("p (c b s) -> p c b s", c=8, b=BS)

        def conv3_oc(oc):
            pss = [cpsum.tile([P, 512], f32, tag="cps", name=f"ps_c3_{oc}_{b}")
                   for b in range(BS)]
            for b in range(BS):
                reg = pss[b][:, 0:S]
                for ic in range(2):
                    nc.tensor.matmul(
                        reg, cw3[:, ((b * 2 + ic) * 8 + oc) * P:
                                 ((b * 2 + ic) * 8 + oc + 1) * P],
                        out2[:, (b * 2 + ic) * S:(b * 2 + ic + 1) * S],
                        start=(ic == 0), stop=False)
                nc.tensor.matmul(
                    reg, ident_sb,
                    x_sb[:, b * 8 * S + oc * S:b * 8 * S + (oc + 1) * S],
                    start=False, stop=True)
            ost = ostage.tile([P, BS * S], f16, tag="ost", name="ost")
            for b in range(BS):
                src = pss[b][:, 0:S]
                epilogue(ost[:, b * S:(b + 1) * S], src, 4 + oc)
            nc.sync.dma_start(out=outd_v[:, oc], in_=ost)

        prev3 = None
        for oc in range(8):
            for ic in range(2):
                bank = oc * 2 + ic
                ps = cmb_bank(3, w3_sb, bd3, bank)
                if ic == 1 and prev3 is not None:
                    conv3_oc(prev3)
                psum_copy(cw3_v[:, :, ic, oc],
                          ps.rearrange("p (gl b j) -> p b gl j", gl=8, b=BS))
            prev3 = oc
        conv3_oc(prev3)

    nc.finalize()
    return nc


# ----------------------------------------------------------------------------
# Entry point
# ----------------------------------------------------------------------------

def kernel(x, w1, w2, w3, r1_w, r1_b, r2_w, r2_b, r3_w, r3_b,
           bn1_g, bn1_b, bn1_m, bn1_v, bn2_g, bn2_b, bn2_m, bn2_v,
           bn3_g, bn3_b, bn3_m, bn3_v, _trace=False):
    global _nc_cache, last_exec_time_ns, last_trace_path, last_res
    from concourse.bass_utils import run_bass_kernel_spmd

    prep = _prep_weights(
        np.asarray(w1, np.float32), np.asarray(w2, np.float32),
        np.asarray(w3, np.float32),
        np.asarray(r1_w, np.float32), np.asarray(r1_b, np.float32),
        np.asarray(r2_w, np.float32), np.asarray(r2_b, np.float32),
        np.asarray(r3_w, np.float32), np.asarray(r3_b, np.float32),
        np.asarray(bn1_g, np.float32), np.asarray(bn1_b, np.float32),
        np.asarray(bn1_m, np.float32), np.asarray(bn1_v, np.float32),
        np.asarray(bn2_g, np.float32), np.asarray(bn2_b, np.float32),
        np.asarray(bn2_m, np.float32), np.asarray(bn2_v, np.float32),
        np.asarray(bn3_g, np.float32), np.asarray(bn3_b, np.float32),
        np.asarray(bn3_m, np.float32), np.asarray(bn3_v, np.float32))
    xs = _prep_x(np.asarray(x, np.float32))

    shared_map = {
        "w1r": prep["w1r"], "w2r": prep["w2r"], "w3r": prep["w3r"],
        "cbf": prep["cbf"], "cf32": prep["cf32"],
    }
    in_maps = [dict(shared_map, x_f16=xs[c]) for c in range(NCORES)]

    if _nc_cache is None:
        _nc_cache = _build_nc()
    res = run_bass_kernel_spmd(_nc_cache, in_maps, core_ids=list(range(NCORES)),
                               trace=_trace)
    last_exec_time_ns = res.exec_time_ns
    last_trace_path = (res.instructions_and_trace or (None, None))[1]
    last_res = res

    out = np.empty((B, OUTP, H, H), np.float32)
    for c in range(NCORES):
        o = np.asarray(res.results[c]["out"], np.float32)   # [128, 8*BS*S]
        out[c * BS:(c + 1) * BS] = (
            o.reshape(P, 8, BS, S).transpose(2, 1, 0, 3).reshape(BS, OUTP, H, H))
    return out


# revision 8
# speedup vs baseline: 1.1859x; 1.1859x over previous
"""Trainium2 Bass kernel for the BottleneckIndependent MoE-routed conv block.

Math (per sample b):
  rw1 = sigmoid(mean_hw(x) @ r1_w + r1_b)                     [E]
  cw1 = sum_e rw1[e] * w1[e]          (per-sample 1x1 weights)
  out1 = relu(bn1(cw1 @ x))
  rw2 / cw2 / out2: same with 3x3 conv (pad 1)
  rw3 / cw3: 1x1; out = relu(bn3(cw3 @ out2) + x)

Strategy (8 cores, data-parallel over batch, 4 samples/core):
  * BN scales fold into expert weights on the host; BN bias + ReLU fuse into
    one ScalarE/VectorE epilogue op per output chunk.  All device tensors are
    bf16.
  * The rank-8 expert combine runs on the PE with the expert weights as the
    STATIONARY operand ([128,128] chunks, rows=(j,e)) against a block-diagonal
    routing matrix bd[128, 64] (cols=(b,j)), yielding combined weights directly
    in [i_partition, ...] layout -- the lhsT layout the conv matmuls need.
  * Single-queue FIFO DMA: every input rides the sync (SP) HWDGE queue in
    consumption order x -> cbf/cf32 -> w1 -> w2 -> w3 with no cross-queue
    dependency chains, so the ~435 GB/s per-core port never idles between
    tensors.  Output writebacks also ride sync (in-order after w3's dispatch).
  * Convs are bank-paced: weight chunk order in DRAM is arranged so each
    combine bank (8 chunks -> one PSUM tile) unblocks exactly the conv matmuls
    that consume it; conv work trails the weight DMA by ~1 bank instead of a
    whole oc-half.  Stage-3 (combine3+conv3+writeback) pipelines per output
    chunk behind w3's 4 slices.
  * The 3x3 is 9 shifted 1x1 matmuls accumulating in PSUM over a zero-padded
    16x16 buffer.  The residual add is an identity matmul into the conv3
    PSUM group.  Pool/bias epilogues alternate ACT/DVE; PSUM->SBUF combine
    copies rotate DVE/ACT/GPSIMD.
"""

import numpy as np
import ml_dtypes

B, INP, WIDTH, OUTP, E, H = 32, 1024, 256, 1024, 8, 14
EPS = 1e-5
S = H * H            # 196
SP = 256             # 16*16 padded spatial
NCORES = 8
BS = B // NCORES     # 4 samples per core
P = 128

BF16 = ml_dtypes.bfloat16

N_CHUNKS1 = 2 * 8 * 8        # stage1: c1 = (oc*8 + ic)*8 + gl   [bank=(oc,ic)]
N_CHUNKS2 = 2 * 9 * 2 * 8    # stage2: c2 = gh*144+tap*16+ic*8+gl [bank=(gh,tap,ic)]
N_CHUNKS3 = 8 * 2 * 8        # stage3: c3 = (oc*2 + ic)*8 + gl   [bank=(oc,ic)]

_nc_cache = None
last_exec_time_ns = None
last_trace_path = None
last_res = None


# ----------------------------------------------------------------------------
# Host-side input preparation (pure numpy)
# ----------------------------------------------------------------------------

def _fold_bn(g, b, m, v):
    inv = (g / np.sqrt(v + EPS)).astype(np.float32)
    beta = (b - m * inv).astype(np.float32)
    return inv, beta


def _prep_weights(w1, w2, w3, r1_w, r1_b, r2_w, r2_b, r3_w, r3_b,
                  bn1_g, bn1_b, bn1_m, bn1_v, bn2_g, bn2_b, bn2_m, bn2_v,
                  bn3_g, bn3_b, bn3_m, bn3_v):
    inv1, beta1 = _fold_bn(bn1_g, bn1_b, bn1_m, bn1_v)
    inv2, beta2 = _fold_bn(bn2_g, bn2_b, bn2_m, bn2_v)
    inv3, beta3 = _fold_bn(bn3_g, bn3_b, bn3_m, bn3_v)

    w1p = (w1[:, :, :, 0, 0] * inv1[None, :, None]).astype(np.float32)  # [E,256,1024]
    w2p = (w2 * inv2[None, :, None, None, None]).astype(np.float32)     # [E,256,256,3,3]
    w3p = (w3[:, :, :, 0, 0] * inv3[None, :, None]).astype(np.float32)  # [E,1024,256]

    # stage1 rows (j,e); chunk c1 = (oc*8+ic)*8+gl; o = (oc*8+gl)*16+j
    a = w1p.reshape(E, 2, 8, 16, 8, P)                  # e, oc, gl, j, ic, ip
    w1r = a.transpose(3, 0, 1, 4, 2, 5).reshape(P, N_CHUNKS1 * P).astype(BF16)

    # stage2 chunk c2 = gh*144 + tap*16 + ic*8 + gl; g = gh*8+gl
    a = w2p.reshape(E, 2, 8, 16, 2, P, 3, 3)            # e, gh, gl, j, ic, ip, kh, kw
    w2r = a.transpose(3, 0, 1, 6, 7, 4, 2, 5).reshape(P, N_CHUNKS2 * P).astype(BF16)

    # stage3 chunk c3 = (oc*2+ic)*8+gl; o = (oc*8+gl)*16+j
    a = w3p.reshape(E, 8, 8, 16, 2, P)                  # e, oc, gl, j, ic, ip
    w3r = a.transpose(3, 0, 1, 4, 2, 5).reshape(P, N_CHUNKS3 * P).astype(BF16)

    def rep_routing(rw, nchunks):
        # [C, E] -> [128, nchunks*128]; col m of chunk ic = rw[ic*128+p, m%8]/S
        r = (np.asarray(rw, np.float32) / float(S)).reshape(nchunks, P, E)
        rrep = np.tile(r[:, :, None, :], (1, 1, 16, 1)).reshape(nchunks, P, P)
        return rrep.transpose(1, 0, 2).reshape(P, nchunks * P)

    # fp16 constant pack: mask(64) | ident(128) | r1rep(1024) | r2rep(256) | r3rep(256)
    jj = np.arange(P)[:, None] // 8
    col_j = np.tile(np.arange(16), 4)[None, :]
    mask = (col_j == jj).astype(np.float32)
    ident = np.eye(P, dtype=np.float32)
    cbf = np.concatenate(
        [mask, ident, rep_routing(r1_w, 8), rep_routing(r2_w, 2),
         rep_routing(r3_w, 2)], axis=1).astype(BF16)    # [128, 1728]

    # f32 constant pack: rb(3) | beta1(2) | beta2(2) | beta3(8)
    rb = np.stack([np.tile(np.asarray(r, np.float32), 16)
                   for r in (r1_b, r2_b, r3_b)], axis=1)
    cf32 = np.concatenate(
        [rb, beta1.reshape(2, P).T, beta2.reshape(2, P).T,
         beta3.reshape(8, P).T], axis=1).astype(np.float32)  # [128, 15]

    return dict(w1r=w1r, w2r=w2r, w3r=w3r, cbf=cbf, cf32=cf32)


def _prep_x(x):
    out = []
    for c in range(NCORES):
        xc = np.asarray(x[c * BS:(c + 1) * BS], np.float32)
        xb = xc.reshape(BS, 8, P, S).transpose(2, 0, 1, 3).reshape(P, BS * 8 * S)
        out.append(np.ascontiguousarray(xb.astype(BF16)))
    return out


# ----------------------------------------------------------------------------
# Device program
# ----------------------------------------------------------------------------

def _build_nc():
    import concourse.tile as tile
    import concourse.mybir as mybir
    from concourse.bacc import Bacc
    from contextlib import ExitStack

    f32 = mybir.dt.float32
    bf16 = mybir.dt.bfloat16
    AF = mybir.ActivationFunctionType
    ALU = mybir.AluOpType
    AX = mybir.AxisListType

    nc = Bacc("TRN2")

    xd = nc.dram_tensor("x_bf", [P, BS * 8 * S], bf16, kind="ExternalInput")
    w1d = nc.dram_tensor("w1r", [P, N_CHUNKS1 * P], bf16, kind="ExternalInput")
    w2d = nc.dram_tensor("w2r", [P, N_CHUNKS2 * P], bf16, kind="ExternalInput")
    w3d = nc.dram_tensor("w3r", [P, N_CHUNKS3 * P], bf16, kind="ExternalInput")
    cbfd = nc.dram_tensor("cbf", [P, 1728], bf16, kind="ExternalInput")
    cf32d = nc.dram_tensor("cf32", [P, 15], f32, kind="ExternalInput")
    # oc-major output: [P, (oc, b, s)]
    outd = nc.dram_tensor("out", [P, 8 * BS * S], bf16, kind="ExternalOutput")

    with tile.TileContext(nc) as tc, ExitStack() as ctx:
        singles = ctx.enter_context(tc.tile_pool(name="singles", bufs=1))
        wbig = ctx.enter_context(tc.tile_pool(name="wbig", bufs=1))
        cwa = ctx.enter_context(tc.tile_pool(name="cwa", bufs=1))
        ostage = ctx.enter_context(tc.tile_pool(name="ostage", bufs=2))
        kpsum = ctx.enter_context(tc.tile_pool(name="kpsum", bufs=3, space="PSUM"))
        cpsum = ctx.enter_context(tc.tile_pool(name="cpsum", bufs=4, space="PSUM"))
        rpsum = ctx.enter_context(tc.tile_pool(name="rpsum", bufs=1, space="PSUM"))

        # ---- single-queue DMA plan: x, cbf, cf32, w1, w2 (w3 after stage1) --
        x_sb = singles.tile([P, BS * 8 * S], bf16)
        for sl in range(2):
            w = BS * 8 * S // 2
            nc.sync.dma_start(out=x_sb[:, sl * w:(sl + 1) * w],
                              in_=xd[:, sl * w:(sl + 1) * w])
        cbf_sb = singles.tile([P, 1728], bf16)
        nc.sync.dma_start(out=cbf_sb, in_=cbfd[:, :])
        cf32_sb = singles.tile([P, 15], f32)
        nc.sync.dma_start(out=cf32_sb, in_=cf32d[:, :])

        mask_sb = cbf_sb[:, 0:64]
        ident_sb = cbf_sb[:, 64:192]
        r1w_sb = cbf_sb[:, 192:1216]
        r2w_sb = cbf_sb[:, 1216:1472]
        r3w_sb = cbf_sb[:, 1472:1728]
        rb_sb = cf32_sb[:, 0:3]
        beta_sb = cf32_sb[:, 3:15]

        w1_sb = wbig.tile([P, N_CHUNKS1 * P], bf16, tag="wbig", name="w1_sb")
        for sl in range(4):
            w = N_CHUNKS1 * P // 4
            nc.sync.dma_start(out=w1_sb[:, sl * w:(sl + 1) * w],
                              in_=w1d[:, sl * w:(sl + 1) * w])
        w2_sb = singles.tile([P, N_CHUNKS2 * P], bf16)
        for sl in range(6):
            w = N_CHUNKS2 * P // 6
            nc.sync.dma_start(out=w2_sb[:, sl * w:(sl + 1) * w],
                              in_=w2d[:, sl * w:(sl + 1) * w])

        # ---- working tiles ----------------------------------------------
        cw1 = cwa.tile([P, BS * 8 * 2 * P], bf16, tag="cwa", name="cw1")
        cw2 = singles.tile([P, BS * 9 * 2 * 2 * P], bf16)
        out1pad = singles.tile([P, BS * 2 * SP], bf16)
        nc.vector.memset(out1pad, 0.0)
        out2 = singles.tile([P, BS * 2 * S], bf16)

        pscratch = singles.tile([P, 8 * S], bf16)
        pool1 = singles.tile([P, 8 * BS], f32)
        pool1b = singles.tile([P, 8 * BS], bf16)
        pool2 = singles.tile([P, 2 * BS], f32)
        pool2b = singles.tile([P, 2 * BS], bf16)
        pool3 = singles.tile([P, 2 * BS], f32)
        pool3b = singles.tile([P, 2 * BS], bf16)

        x_v = x_sb.rearrange("p (b c s) -> p b c s", b=BS, c=8)
        mask_v = mask_sb.rearrange("p (b j) -> p b j", b=BS)
        out1pad_v = out1pad.rearrange("p (b c h w) -> p b c h w", b=BS, c=2, h=16)
        # cw views: cols (b, ic, oc, gl, j)
        cw1_v = cw1.rearrange("p (b ic oc gl j) -> p b ic oc gl j",
                              b=BS, ic=8, oc=2, gl=8)
        cw2_v = cw2.rearrange("p (b t ic gh gl j) -> p gh t ic gl b j",
                              b=BS, t=9, ic=2, gh=2, gl=8)

        # ---- engine rotation helpers -------------------------------------
        cp_i = [0]

        def psum_copy(dst, src):
            k = cp_i[0] % 2
            cp_i[0] += 1
            if k == 0:
                nc.vector.tensor_copy(out=dst, in_=src)
            else:
                nc.scalar.copy(dst, src)

        ep_i = [0]

        def epilogue(dst, src, bias_col):
            k = ep_i[0] % 2
            ep_i[0] += 1
            if k == 0:
                nc.scalar.activation(out=dst, in_=src, func=AF.Relu,
                                     bias=beta_sb[:, bias_col:bias_col + 1],
                                     scale=1.0)
            else:
                nc.vector.tensor_scalar(
                    out=dst, in0=src,
                    scalar1=beta_sb[:, bias_col:bias_col + 1], scalar2=0.0,
                    op0=ALU.add, op1=ALU.max)

        pl_i = [0]

        def pool_piece(dst, src):
            # dst [P,1]; src 2D [P,W]
            k = pl_i[0] % 2
            pl_i[0] += 1
            if k == 0:
                nc.vector.tensor_reduce(out=dst, in_=src, axis=AX.X, op=ALU.add)
            else:
                w = src.shape[-1]
                nc.scalar.activation(out=pscratch[:, 0:w], in_=src,
                                     func=AF.Copy, accum_out=dst)

        # ---- pooling 1 (sum over spatial; 1/S folded into routing w) -----
        pool1_v = pool1.rearrange("p (c b) -> p c b", b=BS)
        for b in range(BS):
            if b % 2 == 0:
                nc.vector.tensor_reduce(out=pool1_v[:, :, b], in_=x_v[:, b],
                                        axis=AX.X, op=ALU.add)
            else:
                for c8 in range(8):
                    nc.scalar.activation(
                        out=pscratch[:, c8 * S:(c8 + 1) * S],
                        in_=x_sb[:, b * 8 * S + c8 * S:b * 8 * S + (c8 + 1) * S],
                        func=AF.Copy, accum_out=pool1_v[:, c8, b:b + 1])
        nc.vector.tensor_copy(out=pool1b, in_=pool1)

        # ---- routing helper ----------------------------------------------
        def routing(st, rw_sb, pool_f16, nchunks):
            ps = rpsum.tile([P, BS], f32, tag="rps", name=f"ps_rt{st}")
            for ic in range(nchunks):
                nc.tensor.matmul(ps, rw_sb[:, ic * P:(ic + 1) * P],
                                 pool_f16[:, ic * BS:(ic + 1) * BS],
                                 start=(ic == 0), stop=(ic == nchunks - 1))
            rwt = singles.tile([P, BS], bf16, name=f"rwt{st}")
            nc.scalar.activation(out=rwt, in_=ps, func=AF.Sigmoid,
                                 bias=rb_sb[:, st:st + 1], scale=1.0)
            bd = singles.tile([P, BS * 16], bf16, name=f"bd{st}")
            bd_v = bd.rearrange("p (b j) -> p b j", b=BS)
            nc.vector.tensor_tensor(
                out=bd_v, in0=mask_v,
                in1=rwt[:, :, None].to_broadcast((P, BS, 16)),
                op=ALU.mult)
            return bd

        def cmb_bank(st, w_sb, bd, bank):
            ps = kpsum.tile([P, 512], f32, tag="kps", name=f"ps_cmb{st}")
            for c8 in range(8):
                c = bank * 8 + c8
                nc.tensor.matmul(ps[:, c8 * 64:(c8 + 1) * 64],
                                 w_sb[:, c * P:(c + 1) * P], bd,
                                 start=True, stop=True)
            return ps

        # ================== stage 1 =======================================
        bd1 = routing(0, r1w_sb, pool1b, 8)

        # bank k = (oc, ic); combine -> copy; conv1 mm for bank k-1 (lag 1)
        conv1_ps = {}

        def conv1_mm(oc, ic):
            if ic == 0 and oc not in conv1_ps:
                conv1_ps[oc] = [cpsum.tile([P, 512], f32, tag="cps",
                                           name=f"ps_c1_{oc}_{b}")
                                for b in range(BS)]
            for b in range(BS):
                reg = conv1_ps[oc][b][:, 0:S]
                nc.tensor.matmul(
                    reg, cw1[:, ((b * 8 + ic) * 2 + oc) * P:
                             ((b * 8 + ic) * 2 + oc + 1) * P],
                    x_sb[:, b * 8 * S + ic * S:b * 8 * S + (ic + 1) * S],
                    start=(ic == 0), stop=(ic == 7))

        def conv1_fin(oc):
            for b in range(BS):
                src = conv1_ps[oc][b][:, 0:S]
                epilogue(out1pad_v[:, b, oc, 1:15, 1:15],
                         src.rearrange("p (h w) -> p h w", h=H), oc)
                pool_piece(pool2[:, oc * BS + b:oc * BS + b + 1],
                           out1pad[:, (b * 2 + oc) * SP:(b * 2 + oc + 1) * SP])

        prev1 = None
        for bank in range(16):
            oc, ic = bank // 8, bank % 8
            ps = cmb_bank(1, w1_sb, bd1, bank)
            if prev1 is not None:
                conv1_mm(*prev1)
                if prev1[1] == 7:
                    conv1_fin(prev1[0])
            psum_copy(cw1_v[:, :, ic, oc],
                      ps.rearrange("p (gl b j) -> p b gl j", gl=8, b=BS))
            prev1 = (oc, ic)
        conv1_mm(*prev1)
        conv1_fin(prev1[0])

        # ---- w3 DMA (sync queue, after w2; WAR on w1 slot via wbig tag) --
        w3_sb = wbig.tile([P, N_CHUNKS3 * P], bf16, tag="wbig", name="w3_sb")
        for sl in range(4):
            w = N_CHUNKS3 * P // 4
            nc.sync.dma_start(out=w3_sb[:, sl * w:(sl + 1) * w],
                              in_=w3d[:, sl * w:(sl + 1) * w])

        nc.vector.tensor_copy(out=pool2b, in_=pool2)

        # ================== stage 2 =======================================
        bd2 = routing(1, r2w_sb, pool2b, 2)

        conv2_ps = {}

        def conv2_mm(gh, tap, ic):
            if tap == 0 and ic == 0:
                conv2_ps[gh] = [cpsum.tile([P, 512], f32, tag="cps",
                                           name=f"ps_c2_{gh}_{b}")
                                for b in range(BS)]
            k = tap * 2 + ic
            kh, kw = tap // 3, tap % 3
            for b in range(BS):
                reg = conv2_ps[gh][b][:, 0:S]
                nc.tensor.matmul(
                    reg.rearrange("p (h w) -> p h w", h=H),
                    cw2[:, (((b * 9 + tap) * 2 + ic) * 2 + gh) * P:
                        (((b * 9 + tap) * 2 + ic) * 2 + gh + 1) * P],
                    out1pad_v[:, b, ic, kh:kh + H, kw:kw + H],
                    start=(k == 0), stop=(k == 17))

        def conv2_fin(gh):
            for b in range(BS):
                src = conv2_ps[gh][b][:, 0:S]
                epilogue(out2[:, (b * 2 + gh) * S:(b * 2 + gh + 1) * S],
                         src, 2 + gh)
                pool_piece(pool3[:, gh * BS + b:gh * BS + b + 1],
                           out2[:, (b * 2 + gh) * S:(b * 2 + gh + 1) * S])

        prev2 = None
        for bank in range(36):
            gh, tap, ic = bank // 18, (bank % 18) // 2, bank % 2
            ps = cmb_bank(2, w2_sb, bd2, bank)
            if prev2 is not None:
                conv2_mm(*prev2)
                if prev2[1] == 8 and prev2[2] == 1:
                    conv2_fin(prev2[0])
            psum_copy(cw2_v[:, gh, tap, ic],
                      ps.rearrange("p (gl b j) -> p gl b j", gl=8, b=BS))
            prev2 = (gh, tap, ic)
        conv2_mm(*prev2)
        conv2_fin(prev2[0])

        nc.vector.tensor_copy(out=pool3b, in_=pool3)

        # ================== stage 3 =======================================
        bd3 = routing(2, r3w_sb, pool3b, 2)
        cw3 = cwa.tile([P, BS * 2 * 8 * P], bf16, tag="cwa", name="cw3")
        cw3_v = cw3.rearrange("p (b ic oc gl j) -> p b ic oc gl j",
                              b=BS, ic=2, oc=8, gl=8)
        outd_v = outd.rearrange("p (c b s) -> p c b s", c=8, b=BS)

        def conv3_oc(oc):
            pss = [cpsum.tile([P, 512], f32, tag="cps", name=f"ps_c3_{oc}_{b}")
                   for b in range(BS)]
            for b in range(BS):
                reg = pss[b][:, 0:S]
                for ic in range(2):
                    nc.tensor.matmul(
                        reg, cw3[:, ((b * 2 + ic) * 8 + oc) * P:
                                 ((b * 2 + ic) * 8 + oc + 1) * P],
                        out2[:, (b * 2 + ic) * S:(b * 2 + ic + 1) * S],
                        start=(ic == 0), stop=False)
                nc.tensor.matmul(
                    reg, ident_sb,
                    x_sb[:, b * 8 * S + oc * S:b * 8 * S + (oc + 1) * S],
                    start=False, stop=True)
            ost = ostage.tile([P, BS * S], bf16, tag="ost", name="ost")
            for b in range(BS):
                src = pss[b][:, 0:S]
                epilogue(ost[:, b * S:(b + 1) * S], src, 4 + oc)
            nc.sync.dma_start(out=outd_v[:, oc], in_=ost)

        prev3 = None
        for oc in range(8):
            for ic in range(2):
                bank = oc * 2 + ic
                ps = cmb_bank(3, w3_sb, bd3, bank)
                if ic == 1 and prev3 is not None:
                    conv3_oc(prev3)
                psum_copy(cw3_v[:, :, ic, oc],
                          ps.rearrange("p (gl b j) -> p b gl j", gl=8, b=BS))
            prev3 = oc
        conv3_oc(prev3)

    nc.finalize()
    return nc


# ----------------------------------------------------------------------------
# Entry point
# ----------------------------------------------------------------------------

def kernel(x, w1, w2, w3, r1_w, r1_b, r2_w, r2_b, r3_w, r3_b,
           bn1_g, bn1_b, bn1_m, bn1_v, bn2_g, bn2_b, bn2_m, bn2_v,
           bn3_g, bn3_b, bn3_m, bn3_v, _trace=False):
    global _nc_cache, last_exec_time_ns, last_trace_path, last_res
    from concourse.bass_utils import run_bass_kernel_spmd

    prep = _prep_weights(
        np.asarray(w1, np.float32), np.asarray(w2, np.float32),
        np.asarray(w3, np.float32),
        np.asarray(r1_w, np.float32), np.asarray(r1_b, np.float32),
        np.asarray(r2_w, np.float32), np.asarray(r2_b, np.float32),
        np.asarray(r3_w, np.float32), np.asarray(r3_b, np.float32),
        np.asarray(bn1_g, np.float32), np.asarray(bn1_b, np.float32),
        np.asarray(bn1_m, np.float32), np.asarray(bn1_v, np.float32),
        np.asarray(bn2_g, np.float32), np.asarray(bn2_b, np.float32),
        np.asarray(bn2_m, np.float32), np.asarray(bn2_v, np.float32),
        np.asarray(bn3_g, np.float32), np.asarray(bn3_b, np.float32),
        np.asarray(bn3_m, np.float32), np.asarray(bn3_v, np.float32))
    xs = _prep_x(np.asarray(x, np.float32))

    shared_map = {
        "w1r": prep["w1r"], "w2r": prep["w2r"], "w3r": prep["w3r"],
        "cbf": prep["cbf"], "cf32": prep["cf32"],
    }
    in_maps = [dict(shared_map, x_bf=xs[c]) for c in range(NCORES)]

    if _nc_cache is None:
        _nc_cache = _build_nc()
    res = run_bass_kernel_spmd(_nc_cache, in_maps, core_ids=list(range(NCORES)),
                               trace=_trace)
    last_exec_time_ns = res.exec_time_ns
    last_trace_path = (res.instructions_and_trace or (None, None))[1]
    last_res = res

    out = np.empty((B, OUTP, H, H), np.float32)
    for c in range(NCORES):
        o = np.asarray(res.results[c]["out"], np.float32)   # [128, 8*BS*S]
        out[c * BS:(c + 1) * BS] = (
            o.reshape(P, 8, BS, S).transpose(2, 1, 0, 3).reshape(BS, OUTP, H, H))
    return out
